# revision 9
# baseline (speedup 1.0000x reference)
"""Trainium2 Bass kernel for nn_DataEmbedding (linear embed + positional + GCN).

out[b,n,t,:] = x[b,n,t,:] @ W_lin + b_lin + pe[t,:] + gcn(emb_table)[n,:]

Sharding: graph-partitioned by destination node. Core k owns nodes
[625k, 625(k+1)) and produces the output shard out[:, 625k:625(k+1), :, :].
No collectives. Host does index/layout prep (PyG-style gcn_norm adjacency
preprocessing, COO->dense packing, padding); the tensor math (the GEMMs and
all per-element work over [B,N,T,D]) runs on device.

v6 design. Measured facts this build exploits: the HAM governor halves the
PE clock at t~56us absolute (other engines and DMA stay full speed), loads
are DMA-descriptor-latency-bound (~400ns/descriptor fixed), and fp8
DoubleRow streams 1.0 cyc/col (no column-rate win), so:
- Loads use few, large descriptors (combined [emb|A'blk0] tile -> 15KB/
  partition in ONE dma_start; A'blk1-4 as one 20KB/partition dma) across
  3 parallel queues (ACT ring, SP ring, GpSimd SWDGE). Stores get the SP
  ring to themselves afterwards.
- GCN (fp8 DoubleRow Y^T passes + all five ve = Y@W_gcn) runs ~10-25us,
  inside the full-clock window; block 0 first so stores start ~20us.
- Main linear: bf16 matmuls, K=38 rows carrying W_lin blocks + pe + fused
  bias (b_lin + b_gcn).
- PSUM->bf16 drains + ve adds spread over ACT/Vector/GpSimd (GpSimd can't
  read PSUM, so it adds after an ACT staging copy).
- Outputs bf16, one contiguous [128,3072] store per (block,batch).
"""

import numpy as np
import ml_dtypes

import concourse.bacc as bacc
import concourse.mybir as mybir
from concourse.bass_utils import run_bass_kernel_spmd
from concourse.tile import TileContext

# problem constants (hardcoded per contract)
B, N, T, CIN, D, E = 8, 5000, 12, 3, 256, 160000
NCORES = 8
NPC = N // NCORES        # real nodes per core = 625
BLK = 128                # nodes per block / output partition tile
NBLK = 5                 # blocks per core; NBLK*BLK = 640 (15 pad rows)
NPAD = NBLK * BLK        # padded local nodes = 640
NT = (N + 127) // 128    # global 128-node src tiles = 40
NPAIR = NT // 2          # DoubleRow k-tile pairs = 20
NG = NT * 128            # padded global nodes = 5120
KX = 3 * T + 2           # main matmul contraction: (t,c) rows + pe + bias = 38
TD = T * D               # 3072
NBB = NBLK * B * BLK     # lhsT total columns = 5120
RXW = TD + NBB           # packed rhs38|x38 width = 8192
AW = NT * BLK            # per-block adjacency width = 5120
ED = NT * D              # emb width = 10240

f32 = mybir.dt.float32
b16 = mybir.dt.bfloat16
f8 = mybir.dt.float8e4

# drain-engine pattern over idx = b*3 + g (24 groups per block). GPSIMD
# cannot access PSUM, so staged groups go through an ACT copy first:
# 'V' = DVE direct add from PSUM, 'A' = ACT copy + DVE bf16 add,
# 'Q' = ACT copy + GpSimd bf16 add
_PAT = (["V", "Q", "A"] * 7) + ["V", "V", "V"]  # 10 V, 7 Q, 7 A

_KERNEL_CACHE: dict = {}


def _pe_table() -> np.ndarray:
    pos = np.arange(T, dtype=np.float32)[:, None]
    div = np.exp(np.arange(0, D, 2, dtype=np.float32) * (-np.log(10000.0) / D))
    pe = np.zeros((T, D), dtype=np.float32)
    pe[:, 0::2] = np.sin(pos * div)
    pe[:, 1::2] = np.cos(pos * div)
    return pe


def _prep(x, edge_index, weights, W_lin, b_lin, b_gcn, emb_table):
    """Host-side sharding/layout prep: gcn_norm (self-loops + symmetric
    normalization, PyG-style cached preprocessing), COO->dense packing with
    duplicate coalescing, padding, matmul operand layout."""
    ei = np.asarray(edge_index).astype(np.int64)
    w = np.asarray(weights, dtype=np.float32)
    loop = np.arange(N, dtype=np.int64)
    row2 = np.concatenate([ei[0], loop])
    col2 = np.concatenate([ei[1], loop])
    w2 = np.concatenate([w, np.ones(N, dtype=np.float32)])
    deg = np.bincount(col2, weights=w2, minlength=N).astype(np.float32)
    dinv = np.zeros_like(deg)
    np.divide(1.0, np.sqrt(deg, where=deg > 0, out=np.ones_like(deg)),
              where=deg > 0, out=dinv)
    wn = dinv[row2] * w2 * dinv[col2]  # normalized edge weights

    # main-matmul rhs [KX, TD]: rows 3t+c carry W_lin[c] in the t-block of
    # columns, row 36 = positional encoding, row 37 = b_lin + b_gcn
    pe = _pe_table()
    rhs38 = np.zeros((KX, TD), dtype=np.float32)
    Wl = np.asarray(W_lin, np.float32)
    for t in range(T):
        for c in range(CIN):
            rhs38[3 * t + c, t * D: (t + 1) * D] = Wl[c]
    rhs38[36] = pe.reshape(-1)
    bfused = np.asarray(b_lin, dtype=np.float32) + np.asarray(b_gcn, np.float32)
    rhs38[37] = np.tile(bfused, T)

    # padded fp8 embedding table, [128, NT*D] partition-major
    emb_pad = np.zeros((NG, D), dtype=np.float32)
    emb_pad[:N] = np.asarray(emb_table, dtype=np.float32)
    emb_pm = np.ascontiguousarray(
        emb_pad.reshape(NT, 128, D).transpose(1, 0, 2).reshape(128, ED)
    ).astype(ml_dtypes.float8_e4m3fn)

    xa = np.asarray(x, dtype=np.float32)
    per_core = []
    for k in range(NCORES):
        lo = k * NPC
        # dense normalized adjacency A_hat[src 5120, dst_local 640] fp8,
        # partition-major: A_pm[p, blk*AW + j*BLK + q] <->
        # A_hat[128j+p, blk*128+q]
        m = (col2 >= lo) & (col2 < lo + NPC)
        A = np.zeros((NG, NPAD), dtype=np.float32)
        np.add.at(A, (row2[m], col2[m] - lo), wn[m])
        A_pm = np.ascontiguousarray(
            A.reshape(NT, 128, NBLK, BLK).transpose(1, 2, 0, 3)
            .reshape(128, NBLK * AW)
        ).astype(ml_dtypes.float8_e4m3fn)

        # x in matmul-ready lhsT layout, packed behind rhs38: K rows are
        # (t,c) pairs then two ones-rows (pe, bias); batches along free dim
        xs = np.zeros((B, NPAD, T, CIN), dtype=np.float32)
        xs[:, :NPC] = xa[:, lo: lo + NPC]
        xs = xs.reshape(B, NBLK, BLK, T, CIN)
        x38 = np.ones((NBLK, KX, B, BLK), dtype=np.float32)
        x38[:, : 3 * T] = xs.transpose(1, 3, 4, 0, 2).reshape(NBLK, 3 * T, B, BLK)
        x38 = x38.reshape(NBLK, KX, B * BLK).transpose(1, 0, 2).reshape(KX, NBB)
        # [emb | A'blk0] interleaved per src tile j: cols j*384+[0:256] =
        # emb[j], [256:384] = A'blk0[j] -- so each load chunk carries both
        # gcn operands for a j range
        eA0 = np.concatenate(
            [emb_pm.reshape(128, NT, D),
             A_pm.reshape(128, NBLK, NT, BLK)[:, 0].reshape(128, NT, BLK)],
            axis=2).reshape(128, NT * (D + BLK))
        # A'blk1-4 j-major: cols j*(4*BLK) + (blk-1)*BLK + q
        A14 = np.ascontiguousarray(
            A_pm.reshape(128, NBLK, NT, BLK)[:, 1:].transpose(0, 2, 1, 3)
            .reshape(128, NT * 4 * BLK))
        per_core.append(
            {
                "embA0": np.ascontiguousarray(eA0),
                "A14": A14,
                "rx38": np.ascontiguousarray(
                    np.concatenate([rhs38, x38], axis=1)
                ).astype(ml_dtypes.bfloat16),
            }
        )
    return per_core


def _build_kernel():
    if "nc" in _KERNEL_CACHE:
        return _KERNEL_CACHE["nc"]

    nc = bacc.Bacc(num_swdge_queues=4)
    rx_d = nc.declare_dram_parameter("rx38", [KX, RXW], b16, isOutput=False)
    ea_d = nc.declare_dram_parameter("embA0", [128, ED + AW], f8, isOutput=False)
    a14_d = nc.declare_dram_parameter("A14", [128, 4 * AW], f8, isOutput=False)
    wg_d = nc.declare_dram_parameter("W_gcn", [128, 2 * D], b16, isOutput=False)
    # device-chosen layout; host reassembles [B, 625, T, D] from [blk, b, p, td]
    out_d = nc.declare_dram_parameter("out", [NBLK, B, BLK, TD], b16, isOutput=True)

    with TileContext(nc) as tc:
        with (
            tc.tile_pool(name="keep", bufs=1) as kp,
            tc.tile_pool(name="stg", bufs=6) as stgp,
            tc.tile_pool(name="outp", bufs=6) as outp,
            tc.tile_pool(name="gpp", bufs=1, space="PSUM") as gpp,
            tc.tile_pool(name="mps", bufs=3, space="PSUM") as mps,
        ):
            rx = kp.tile([KX, RXW], b16)
            rhs38b = rx[:, 0:TD]
            xall = rx[:, TD:RXW]
            wgb = kp.tile([128, 2 * D], b16)       # W_gcn halves, bf16
            eA0 = kp.tile([128, ED + AW], f8)      # [emb | A'blk0], j-interleaved
            A14 = kp.tile([128, 4 * AW], f8)       # A'blk1-4, j-major
            yT = kp.tile([128, 2 * NPAD], b16)     # [d_half, dh*640 + dst]
            veb = kp.tile([128, NBLK * D], b16)    # ve per block, bf16

            # loads: 3 parallel queues, chunked so gcn matmuls chase the
            # arriving j ranges. ACT ring: [emb|A'blk0] in 4 j-chunks (gcn
            # block-0 critical); SP ring: rx38 + W_gcn (frees early for the
            # stores); GpSimd SWDGE: A'blk1-4 in 2 j-chunks.
            EAW = (D + BLK) * NT // 4
            for c in range(4):
                nc.scalar.dma_start(out=eA0[:, c * EAW: (c + 1) * EAW],
                                    in_=ea_d[:, c * EAW: (c + 1) * EAW])
            nc.sync.dma_start(out=rx[:], in_=rx_d[:])
            nc.sync.dma_start(out=wgb[:], in_=wg_d[:])
            for c in range(2):
                nc.gpsimd.dma_start(out=A14[:, c * 2 * AW: (c + 1) * 2 * AW],
                                    in_=a14_d[:, c * 2 * AW: (c + 1) * 2 * AW])

            eav = eA0[:].rearrange("p (j e) -> p j e", e=D + BLK)
            embsv = eav[:, :, 0:D]
            # A'blk1-4 as [p, j, blk, q] view for the multi-block gcn pass
            A4 = A14[:].rearrange("p (j bl q) -> p j bl q", bl=NBLK - 1, q=BLK)

            gp = gpp.tile([128, 1024], f32, space="PSUM", tag="gp")

            # ---- GCN block 0: Y^T[d, dst0] (fp8 DoubleRow, 2 src k-tiles
            # per matmul; dh halves in separate PSUM banks) ----
            Ab0 = eav[:, :, D: D + BLK]
            for dh in range(2):
                for j2 in range(NPAIR):
                    nc.tensor.matmul(
                        gp[:, dh * 512: dh * 512 + BLK],
                        lhsT=embsv[:, 2 * j2: 2 * j2 + 2,
                                   dh * 128: (dh + 1) * 128],
                        rhs=Ab0[:, 2 * j2: 2 * j2 + 2, :],
                        start=(j2 == 0), stop=(j2 == NPAIR - 1),
                        perf_mode=mybir.MatmulPerfMode.DoubleRow,
                    )
            nc.vector.tensor_copy(
                yT[:].rearrange("p (k c) -> p k c", c=NPAD)[:, :, 0:BLK],
                gp[:].rearrange("p (k c) -> p k c", c=512)[:, :, 0:BLK],
            )
            # ve block 0 (reuses bank 0 of gp after the yT drain)
            for dh in range(2):
                nc.tensor.matmul(
                    gp[:, 0:D],
                    lhsT=yT[:, dh * NPAD: dh * NPAD + BLK],
                    rhs=wgb[:, dh * D: (dh + 1) * D],
                    start=(dh == 0), stop=(dh == 1),
                )
            nc.vector.tensor_copy(veb[:, 0:D], gp[:, 0:D])

            # ---- GCN blocks 1-4 in one 512-dst pass (weight loads
            # amortized over 4 blocks) ----
            for dh in range(2):
                for j2 in range(NPAIR):
                    nc.tensor.matmul(
                        gp[:, dh * 512: (dh + 1) * 512],
                        lhsT=embsv[:, 2 * j2: 2 * j2 + 2,
                                   dh * 128: (dh + 1) * 128],
                        rhs=A4[:, 2 * j2: 2 * j2 + 2, :, :],
                        start=(j2 == 0), stop=(j2 == NPAIR - 1),
                        perf_mode=mybir.MatmulPerfMode.DoubleRow,
                    )
            nc.vector.tensor_copy(
                yT[:].rearrange("p (k c) -> p k c", c=NPAD)[:, :, BLK:NPAD],
                gp[:].rearrange("p (k c) -> p k c", c=512),
            )
            # ve blocks 1-4
            for blk in range(1, NBLK):
                for dh in range(2):
                    nc.tensor.matmul(
                        gp[:, 0:D],
                        lhsT=yT[:, dh * NPAD + blk * BLK:
                                dh * NPAD + (blk + 1) * BLK],
                        rhs=wgb[:, dh * D: (dh + 1) * D],
                        start=(dh == 0), stop=(dh == 1),
                    )
                nc.vector.tensor_copy(veb[:, blk * D: (blk + 1) * D],
                                      gp[:, 0:D])

            # ---- main loop: per (block, batch): 6 matmuls + drains+ve adds
            # + one contiguous store ----
            for blk in range(NBLK):
                ve4 = (
                    veb[:, blk * D: (blk + 1) * D]
                    .rearrange("p d -> p () d").to_broadcast([BLK, 4, D])
                )
                for b in range(B):
                    lhsT = xall[:, (blk * B + b) * BLK: (blk * B + b + 1) * BLK]
                    osb = outp.tile([BLK, TD], b16, tag="osb")
                    for g in range(3):
                        mp = mps.tile([BLK, 1024], f32, space="PSUM", tag="mp")
                        for i in range(2):
                            tp = g * 2 + i  # t-pair index
                            nc.tensor.matmul(
                                mp[:, i * 512: (i + 1) * 512],
                                lhsT=lhsT,
                                rhs=rhs38b[:, tp * 512: (tp + 1) * 512],
                                start=True, stop=True,
                            )
                        oseg = osb[:, g * 1024: (g + 1) * 1024].rearrange(
                            "p (t d) -> p t d", d=D
                        )
                        kind = _PAT[b * 3 + g]
                        if kind == "V":
                            nc.vector.tensor_tensor(
                                out=oseg,
                                in0=mp[:].rearrange("p (t d) -> p t d", d=D),
                                in1=ve4, op=mybir.AluOpType.add,
                            )
                        else:
                            stg = stgp.tile([BLK, 1024], b16, tag="stg")
                            nc.scalar.copy(stg[:], mp[:])
                            eng = nc.gpsimd if kind == "Q" else nc.vector
                            eng.tensor_tensor(
                                out=oseg,
                                in0=stg[:].rearrange("p (t d) -> p t d", d=D),
                                in1=ve4, op=mybir.AluOpType.add,
                            )
                    nc.sync.dma_start(out=out_d[blk, b], in_=osb[:])

    nc.finalize()
    _KERNEL_CACHE["nc"] = nc
    return nc


LAST_RESULTS = None  # BassKernelResults of the most recent run (for profiling)


def kernel(x, x_mark, edge_index, weights, W_lin, b_lin, emb_table, W_gcn, b_gcn):
    global LAST_RESULTS
    per_core = _prep(x, edge_index, weights, W_lin, b_lin, b_gcn, emb_table)
    nc = _build_kernel()
    shared = {
        "W_gcn": np.ascontiguousarray(
            np.asarray(W_gcn, dtype=np.float32).reshape(2, 128, D)
            .transpose(1, 0, 2).reshape(128, 2 * D)).astype(ml_dtypes.bfloat16),
    }
    in_maps = [{**shared, **pc} for pc in per_core]
    res = run_bass_kernel_spmd(nc, in_maps, list(range(NCORES)))
    LAST_RESULTS = res
    shards = []
    for k in range(NCORES):
        o = np.asarray(res.results[k]["out"]).astype(np.float32)
        # [NBLK, B, BLK, TD] -> [B, NPAD, T, D] -> drop pad rows
        o = o.transpose(1, 0, 2, 3).reshape(B, NPAD, T, D)[:, :NPC]
        shards.append(o)
    return np.concatenate(shards, axis=1)


# revision 10
# speedup vs baseline: 1.1783x; 1.1783x over previous
"""Trainium2 Bass kernel for nn_DataEmbedding (linear embed + positional + GCN).

out[b,n,t,:] = x[b,n,t,:] @ W_lin + b_lin + pe[t,:] + gcn(emb_table)[n,:]

Sharding: graph-partitioned by destination node. Core k owns nodes
[625k, 625(k+1)) and produces the output shard out[:, 625k:625(k+1), :, :].
No collectives. Host does index/layout prep (PyG-style gcn_norm adjacency
preprocessing, COO->dense packing, padding); the tensor math (the GEMMs and
all per-element work over [B,N,T,D]) runs on device.

v6 design. Measured facts this build exploits: the HAM governor halves the
PE clock at t~56us absolute (other engines and DMA stay full speed), loads
are DMA-descriptor-latency-bound (~400ns/descriptor fixed), and fp8
DoubleRow streams 1.0 cyc/col (no column-rate win), so:
- Loads use few, large descriptors (combined [emb|A'blk0] tile -> 15KB/
  partition in ONE dma_start; A'blk1-4 as one 20KB/partition dma) across
  3 parallel queues (ACT ring, SP ring, GpSimd SWDGE). Stores get the SP
  ring to themselves afterwards.
- GCN (fp8 DoubleRow Y^T passes + all five ve = Y@W_gcn) runs ~10-25us,
  inside the full-clock window; block 0 first so stores start ~20us.
- Main linear: bf16 matmuls, K=38 rows carrying W_lin blocks + pe + fused
  bias (b_lin + b_gcn).
- PSUM->bf16 drains + ve adds spread over ACT/Vector/GpSimd (GpSimd can't
  read PSUM, so it adds after an ACT staging copy).
- Outputs bf16, one contiguous [128,3072] store per (block,batch).
"""

import numpy as np
import ml_dtypes

import concourse.bacc as bacc
import concourse.mybir as mybir
from concourse.bass_utils import run_bass_kernel_spmd
from concourse.tile import TileContext

# problem constants (hardcoded per contract)
B, N, T, CIN, D, E = 8, 5000, 12, 3, 256, 160000
NCORES = 8
NPC = N // NCORES        # real nodes per core = 625
BLK = 128                # nodes per block / output partition tile
NBLK = 5                 # blocks per core; NBLK*BLK = 640 (15 pad rows)
NPAD = NBLK * BLK        # padded local nodes = 640
NT = (N + 127) // 128    # global 128-node src tiles = 40
NPAIR = NT // 2          # DoubleRow k-tile pairs = 20
NG = NT * 128            # padded global nodes = 5120
KX = 3 * T + 2           # main matmul contraction: (t,c) rows + pe + bias = 38
TD = T * D               # 3072
NBB = NBLK * B * BLK     # lhsT total columns = 5120
RXW = TD + NBB           # packed rhs38|x38 width = 8192
AW = NT * BLK            # per-block adjacency width = 5120
ED = NT * D              # emb width = 10240

f32 = mybir.dt.float32
b16 = mybir.dt.bfloat16
f8 = mybir.dt.float8e4

# drain-engine pattern over idx = b*3 + g (24 groups per block). GPSIMD
# cannot access PSUM, so staged groups go through an ACT copy first:
# 'V' = DVE direct add from PSUM, 'A' = ACT copy + DVE bf16 add,
# 'Q' = ACT copy + GpSimd bf16 add
_PAT = (["V", "Q", "A"] * 7) + ["V", "V", "V"]  # 10 V, 7 Q, 7 A

_KERNEL_CACHE: dict = {}


def _pe_table() -> np.ndarray:
    pos = np.arange(T, dtype=np.float32)[:, None]
    div = np.exp(np.arange(0, D, 2, dtype=np.float32) * (-np.log(10000.0) / D))
    pe = np.zeros((T, D), dtype=np.float32)
    pe[:, 0::2] = np.sin(pos * div)
    pe[:, 1::2] = np.cos(pos * div)
    return pe


def _prep(x, edge_index, weights, W_lin, b_lin, b_gcn, emb_table):
    """Host-side sharding/layout prep: gcn_norm (self-loops + symmetric
    normalization, PyG-style cached preprocessing), COO->dense packing with
    duplicate coalescing, padding, matmul operand layout."""
    ei = np.asarray(edge_index).astype(np.int64)
    w = np.asarray(weights, dtype=np.float32)
    loop = np.arange(N, dtype=np.int64)
    row2 = np.concatenate([ei[0], loop])
    col2 = np.concatenate([ei[1], loop])
    w2 = np.concatenate([w, np.ones(N, dtype=np.float32)])
    deg = np.bincount(col2, weights=w2, minlength=N).astype(np.float32)
    dinv = np.zeros_like(deg)
    np.divide(1.0, np.sqrt(deg, where=deg > 0, out=np.ones_like(deg)),
              where=deg > 0, out=dinv)
    wn = dinv[row2] * w2 * dinv[col2]  # normalized edge weights

    # main-matmul rhs [KX, TD]: rows 3t+c carry W_lin[c] in the t-block of
    # columns, row 36 = positional encoding, row 37 = b_lin + b_gcn
    pe = _pe_table()
    rhs38 = np.zeros((KX, TD), dtype=np.float32)
    Wl = np.asarray(W_lin, np.float32)
    for t in range(T):
        for c in range(CIN):
            rhs38[3 * t + c, t * D: (t + 1) * D] = Wl[c]
    rhs38[36] = pe.reshape(-1)
    bfused = np.asarray(b_lin, dtype=np.float32) + np.asarray(b_gcn, np.float32)
    rhs38[37] = np.tile(bfused, T)

    # padded fp8 embedding table, [128, NT*D] partition-major
    emb_pad = np.zeros((NG, D), dtype=np.float32)
    emb_pad[:N] = np.asarray(emb_table, dtype=np.float32)
    emb_pm = np.ascontiguousarray(
        emb_pad.reshape(NT, 128, D).transpose(1, 0, 2).reshape(128, ED)
    ).astype(ml_dtypes.float8_e4m3fn)

    xa = np.asarray(x, dtype=np.float32)
    per_core = []
    for k in range(NCORES):
        lo = k * NPC
        # dense normalized adjacency A_hat[src 5120, dst_local 640] fp8,
        # partition-major: A_pm[p, blk*AW + j*BLK + q] <->
        # A_hat[128j+p, blk*128+q]
        m = (col2 >= lo) & (col2 < lo + NPC)
        A = np.zeros((NG, NPAD), dtype=np.float32)
        np.add.at(A, (row2[m], col2[m] - lo), wn[m])
        A_pm = np.ascontiguousarray(
            A.reshape(NT, 128, NBLK, BLK).transpose(1, 2, 0, 3)
            .reshape(128, NBLK * AW)
        ).astype(ml_dtypes.float8_e4m3fn)

        # x in matmul-ready lhsT layout, packed behind rhs38: K rows are
        # (t,c) pairs then two ones-rows (pe, bias); batches along free dim
        xs = np.zeros((B, NPAD, T, CIN), dtype=np.float32)
        xs[:, :NPC] = xa[:, lo: lo + NPC]
        xs = xs.reshape(B, NBLK, BLK, T, CIN)
        x38 = np.ones((NBLK, KX, B, BLK), dtype=np.float32)
        x38[:, : 3 * T] = xs.transpose(1, 3, 4, 0, 2).reshape(NBLK, 3 * T, B, BLK)
        x38 = x38.reshape(NBLK, KX, B * BLK).transpose(1, 0, 2).reshape(KX, NBB)
        # [emb | A'blk0] combined tile (emb cols then A'blk0 cols)
        eA0 = np.concatenate(
            [emb_pm, A_pm.reshape(128, NBLK, NT, BLK)[:, 0]
             .reshape(128, NT * BLK)], axis=1)
        # A'blk1-4 j-major: cols j*(4*BLK) + (blk-1)*BLK + q
        A14 = np.ascontiguousarray(
            A_pm.reshape(128, NBLK, NT, BLK)[:, 1:].transpose(0, 2, 1, 3)
            .reshape(128, NT * 4 * BLK))
        per_core.append(
            {
                "embA0": np.ascontiguousarray(eA0),
                "A14": A14,
                "rx38": np.ascontiguousarray(
                    np.concatenate([rhs38, x38], axis=1)
                ).astype(ml_dtypes.bfloat16),
            }
        )
    return per_core


def _build_kernel():
    if "nc" in _KERNEL_CACHE:
        return _KERNEL_CACHE["nc"]

    nc = bacc.Bacc(num_swdge_queues=4)
    rx_d = nc.declare_dram_parameter("rx38", [KX, RXW], b16, isOutput=False)
    ea_d = nc.declare_dram_parameter("embA0", [128, ED + AW], f8, isOutput=False)
    a14_d = nc.declare_dram_parameter("A14", [128, 4 * AW], f8, isOutput=False)
    wg_d = nc.declare_dram_parameter("W_gcn", [128, 2 * D], b16, isOutput=False)
    # device-chosen layout; host reassembles [B, 625, T, D] from [blk, b, p, td]
    out_d = nc.declare_dram_parameter("out", [NBLK, B, BLK, TD], b16, isOutput=True)

    with TileContext(nc) as tc:
        with (
            tc.tile_pool(name="keep", bufs=1) as kp,
            tc.tile_pool(name="stg", bufs=6) as stgp,
            tc.tile_pool(name="outp", bufs=6) as outp,
            tc.tile_pool(name="gpp", bufs=1, space="PSUM") as gpp,
            tc.tile_pool(name="mps", bufs=3, space="PSUM") as mps,
        ):
            rx = kp.tile([KX, RXW], b16)
            rhs38b = rx[:, 0:TD]
            xall = rx[:, TD:RXW]
            wgb = kp.tile([128, 2 * D], b16)       # W_gcn halves, bf16
            eA0 = kp.tile([128, ED + AW], f8)      # [emb | A'blk0], j-interleaved
            A14 = kp.tile([128, 4 * AW], f8)       # A'blk1-4, j-major
            yT = kp.tile([128, 2 * NPAD], b16)     # [d_half, dh*640 + dst]
            veb = kp.tile([128, NBLK * D], b16)    # ve per block, bf16

            # loads: critical [emb|A'blk0] bytes split across the ACT and
            # GpSimd queues so they complete first; A'blk1-4 queued BEHIND
            # them on the same queues (FIFO = priority); rx38 + W_gcn on the
            # SP ring, which then frees up for the output stores. All
            # dispatches precede any compute on their queues.
            EH = (ED + AW) // 2
            nc.scalar.dma_start(out=eA0[:, 0:EH], in_=ea_d[:, 0:EH])
            nc.gpsimd.dma_start(out=eA0[:, EH:], in_=ea_d[:, EH:])
            nc.sync.dma_start(out=rx[:], in_=rx_d[:])
            nc.sync.dma_start(out=wgb[:], in_=wg_d[:])
            nc.scalar.dma_start(out=A14[:, 0: 2 * AW], in_=a14_d[:, 0: 2 * AW])
            nc.gpsimd.dma_start(out=A14[:, 2 * AW:], in_=a14_d[:, 2 * AW:])

            embsv = eA0[:, 0:ED].rearrange("p (j d) -> p j d", d=D)
            # A'blk1-4 as [p, j, blk, q] view for the multi-block gcn pass
            A4 = A14[:].rearrange("p (j bl q) -> p j bl q", bl=NBLK - 1, q=BLK)

            gp = gpp.tile([128, 1024], f32, space="PSUM", tag="gp")

            # ---- GCN block 0: Y^T[d, dst0] (fp8 DoubleRow, 2 src k-tiles
            # per matmul; dh halves in separate PSUM banks) ----
            Ab0 = eA0[:, ED:].rearrange("p (j q) -> p j q", q=BLK)
            for dh in range(2):
                for j2 in range(NPAIR):
                    nc.tensor.matmul(
                        gp[:, dh * 512: dh * 512 + BLK],
                        lhsT=embsv[:, 2 * j2: 2 * j2 + 2,
                                   dh * 128: (dh + 1) * 128],
                        rhs=Ab0[:, 2 * j2: 2 * j2 + 2, :],
                        start=(j2 == 0), stop=(j2 == NPAIR - 1),
                        perf_mode=mybir.MatmulPerfMode.DoubleRow,
                    )
            nc.vector.tensor_copy(
                yT[:].rearrange("p (k c) -> p k c", c=NPAD)[:, :, 0:BLK],
                gp[:].rearrange("p (k c) -> p k c", c=512)[:, :, 0:BLK],
            )
            # ve block 0 (reuses bank 0 of gp after the yT drain)
            for dh in range(2):
                nc.tensor.matmul(
                    gp[:, 0:D],
                    lhsT=yT[:, dh * NPAD: dh * NPAD + BLK],
                    rhs=wgb[:, dh * D: (dh + 1) * D],
                    start=(dh == 0), stop=(dh == 1),
                )
            nc.vector.tensor_copy(veb[:, 0:D], gp[:, 0:D])

            # ---- GCN blocks 1-4 in one 512-dst pass (weight loads
            # amortized over 4 blocks) ----
            for dh in range(2):
                for j2 in range(NPAIR):
                    nc.tensor.matmul(
                        gp[:, dh * 512: (dh + 1) * 512],
                        lhsT=embsv[:, 2 * j2: 2 * j2 + 2,
                                   dh * 128: (dh + 1) * 128],
                        rhs=A4[:, 2 * j2: 2 * j2 + 2, :, :],
                        start=(j2 == 0), stop=(j2 == NPAIR - 1),
                        perf_mode=mybir.MatmulPerfMode.DoubleRow,
                    )
            nc.vector.tensor_copy(
                yT[:].rearrange("p (k c) -> p k c", c=NPAD)[:, :, BLK:NPAD],
                gp[:].rearrange("p (k c) -> p k c", c=512),
            )
            # ve blocks 1-4
            for blk in range(1, NBLK):
                for dh in range(2):
                    nc.tensor.matmul(
                        gp[:, 0:D],
                        lhsT=yT[:, dh * NPAD + blk * BLK:
                                dh * NPAD + (blk + 1) * BLK],
                        rhs=wgb[:, dh * D: (dh + 1) * D],
                        start=(dh == 0), stop=(dh == 1),
                    )
                nc.vector.tensor_copy(veb[:, blk * D: (blk + 1) * D],
                                      gp[:, 0:D])

            # ---- main loop: per (block, batch): 6 matmuls + drains+ve adds
            # + one contiguous store ----
            for blk in range(NBLK):
                ve4 = (
                    veb[:, blk * D: (blk + 1) * D]
                    .rearrange("p d -> p () d").to_broadcast([BLK, 4, D])
                )
                for b in range(B):
                    lhsT = xall[:, (blk * B + b) * BLK: (blk * B + b + 1) * BLK]
                    osb = outp.tile([BLK, TD], b16, tag="osb")
                    for g in range(3):
                        mp = mps.tile([BLK, 1024], f32, space="PSUM", tag="mp")
                        for i in range(2):
                            tp = g * 2 + i  # t-pair index
                            nc.tensor.matmul(
                                mp[:, i * 512: (i + 1) * 512],
                                lhsT=lhsT,
                                rhs=rhs38b[:, tp * 512: (tp + 1) * 512],
                                start=True, stop=True,
                            )
                        oseg = osb[:, g * 1024: (g + 1) * 1024].rearrange(
                            "p (t d) -> p t d", d=D
                        )
                        kind = _PAT[b * 3 + g]
                        if kind == "V":
                            nc.vector.tensor_tensor(
                                out=oseg,
                                in0=mp[:].rearrange("p (t d) -> p t d", d=D),
                                in1=ve4, op=mybir.AluOpType.add,
                            )
                        else:
                            stg = stgp.tile([BLK, 1024], b16, tag="stg")
                            nc.scalar.copy(stg[:], mp[:])
                            eng = nc.gpsimd if kind == "Q" else nc.vector
                            eng.tensor_tensor(
                                out=oseg,
                                in0=stg[:].rearrange("p (t d) -> p t d", d=D),
                                in1=ve4, op=mybir.AluOpType.add,
                            )
                    nc.sync.dma_start(out=out_d[blk, b], in_=osb[:])

    nc.finalize()
    _KERNEL_CACHE["nc"] = nc
    return nc


LAST_RESULTS = None  # BassKernelResults of the most recent run (for profiling)


def kernel(x, x_mark, edge_index, weights, W_lin, b_lin, emb_table, W_gcn, b_gcn):
    global LAST_RESULTS
    per_core = _prep(x, edge_index, weights, W_lin, b_lin, b_gcn, emb_table)
    nc = _build_kernel()
    shared = {
        "W_gcn": np.ascontiguousarray(
            np.asarray(W_gcn, dtype=np.float32).reshape(2, 128, D)
            .transpose(1, 0, 2).reshape(128, 2 * D)).astype(ml_dtypes.bfloat16),
    }
    in_maps = [{**shared, **pc} for pc in per_core]
    res = run_bass_kernel_spmd(nc, in_maps, list(range(NCORES)))
    LAST_RESULTS = res
    shards = []
    for k in range(NCORES):
        o = np.asarray(res.results[k]["out"]).astype(np.float32)
        # [NBLK, B, BLK, TD] -> [B, NPAD, T, D] -> drop pad rows
        o = o.transpose(1, 0, 2, 3).reshape(B, NPAD, T, D)[:, :NPC]
        shards.append(o)
    return np.concatenate(shards, axis=1)
